# revision 10
# baseline (speedup 1.0000x reference)
"""Trainium2 Bass kernel for the dynamic-attention-block CNN (nn_DAB).

Data-parallel over batch: 8 samples -> 8 NeuronCores. Each core runs the
full per-sample network with activations resident in SBUF as bf16 padded
"frames": 128 partitions = 64 channels x 2 image halves, each half a
98x196 zero-padded row-major frame (rows -1..96 / 95..192 of the 192x192
image, cols -2..193). 3x3 convs are 9 shift-tap matmuls accumulated in
PSUM; top/bottom halves run in opposite 64x64 quadrants of the PE array
concurrently. Depthwise dynamic convs use host-built diagonal weights.
Tiny d-dependent tensors (dw kernels, CA gates, additive map) are
precomputed on host in numpy.
"""

import sys

for _p in ("/opt/trn_rl_repo", "/root/.axon_site/_ro/pypackages"):
    if _p not in sys.path:
        sys.path.insert(0, _p)

import numpy as np
import ml_dtypes

BF16 = ml_dtypes.bfloat16

B, C, H, W = 8, 64, 192, 192
HW = H * W
FR, FC = 98, 196          # frame rows / cols per half
FF = FR * FC              # frame floats per partition
Q0 = 1 * FC + 2           # first interior frame position (row 1, col 2)
QL = 96 * FC + 194 - Q0   # sweep length covering all interior rows
TILE = 512
ALPHA = 0.1               # leaky slope

# bias pack columns
BI_B1, BI_B2, BI_B3, BI_CB1, BI_CB2, BI_AT1, BI_AT2, BI_AL, BI_Z = range(9)

_CACHE = {}


def _qtiles():
    out = []
    q = Q0
    while q < Q0 + QL:
        n = min(TILE, Q0 + QL - q)
        out.append((q, n))
        q += n
    return out


def _deltas():
    return [(dy - 1) * FC + (dx - 1) for dy in range(3) for dx in range(3)]


def _build_nc():
    import concourse.bacc as bacc
    import concourse.mybir as mybir
    from concourse import tile

    f32 = mybir.dt.float32
    bf16 = mybir.dt.bfloat16
    AF = mybir.ActivationFunctionType
    ALU = mybir.AluOpType

    nc = bacc.Bacc("TRN2", target_bir_lowering=False, debug=False, num_devices=8)

    x_d = nc.dram_tensor("x", [C, HW], f32, kind="ExternalInput").ap()
    w1_d = nc.dram_tensor("w1", [128, 9, 64], bf16, kind="ExternalInput").ap()
    w2_d = nc.dram_tensor("w2", [128, 9, 64], bf16, kind="ExternalInput").ap()
    w3_d = nc.dram_tensor("w3", [128, 9, 64], bf16, kind="ExternalInput").ap()
    kd1_d = nc.dram_tensor("kd1", [128, 9, 64], bf16, kind="ExternalInput").ap()
    kd2_d = nc.dram_tensor("kd2", [128, 9, 64], bf16, kind="ExternalInput").ap()
    cw1_d = nc.dram_tensor("cw1", [128, 64], bf16, kind="ExternalInput").ap()
    cw2_d = nc.dram_tensor("cw2", [128, 64], bf16, kind="ExternalInput").ap()
    bias_d = nc.dram_tensor("bias", [128, 9], f32, kind="ExternalInput").ap()
    a32_d = nc.dram_tensor("a32s", [128, 512], f32, kind="ExternalInput").ap()
    kv1_d = nc.dram_tensor("kv1", [128, 9], f32, kind="ExternalInput").ap()
    kv2_d = nc.dram_tensor("kv2", [128, 9], f32, kind="ExternalInput").ap()
    y_d = nc.dram_tensor("y", [C, HW], f32, kind="ExternalOutput").ap()

    qt = _qtiles()
    dl = _deltas()

    from contextlib import ExitStack
    with tile.TileContext(nc) as tc, ExitStack() as ctx:
        wpool = ctx.enter_context(tc.tile_pool(name="w", bufs=1))
        xpool = ctx.enter_context(tc.tile_pool(name="x", bufs=1))
        mpool = ctx.enter_context(tc.tile_pool(name="maps", bufs=2))
        stg = ctx.enter_context(tc.tile_pool(name="stg", bufs=3))
        t1p = ctx.enter_context(tc.tile_pool(name="t1", bufs=3))
        t3p = ctx.enter_context(tc.tile_pool(name="t3", bufs=3))
        outp = ctx.enter_context(tc.tile_pool(name="outp", bufs=3))
        psA = ctx.enter_context(tc.tile_pool(name="psA", bufs=5, space="PSUM"))
        accp = ctx.enter_context(tc.tile_pool(name="acc", bufs=4))
        psB = ctx.enter_context(tc.tile_pool(name="psB", bufs=3, space="PSUM"))

        # ---- weights / constants to SBUF ----
        w1 = wpool.tile([128, 9, 64], bf16, tag="w1")
        w2 = wpool.tile([128, 9, 64], bf16, tag="w2")
        w3 = wpool.tile([128, 9, 64], bf16, tag="w3")
        kd1 = wpool.tile([128, 9, 64], bf16, tag="kd1")
        kd2 = wpool.tile([128, 9, 64], bf16, tag="kd2")
        cw1 = wpool.tile([128, 64], bf16, tag="cw1")
        cw2 = wpool.tile([128, 64], bf16, tag="cw2")
        bias = wpool.tile([128, 9], f32, tag="bias")
        kv1 = wpool.tile([128, 9], f32, tag="kv1")
        kv2 = wpool.tile([128, 9], f32, tag="kv2")
        a32 = wpool.tile([128, 512], f32, tag="a32")
        asm = wpool.tile([128, 16, 32, 6], bf16, tag="asm")
        for t, d in ((w1, w1_d), (w2, w2_d), (w3, w3_d), (kd1, kd1_d),
                     (kd2, kd2_d), (cw1, cw1_d), (cw2, cw2_d), (bias, bias_d),
                     (a32, a32_d), (kv1, kv1_d), (kv2, kv2_d)):
            nc.gpsimd.dma_start(out=t[:], in_=d)

        alpha_ap = bias[:, BI_AL:BI_AL + 1]
        zero_ap = bias[:, BI_Z:BI_Z + 1]

        # additive map, column-expanded (rows expand via step-0 AP later)
        nc.vector.tensor_copy(
            asm[:, :, :, :],
            a32[:, :].rearrange("p (g c) -> p g c", g=16).unsqueeze(3)
            .broadcast_to((128, 16, 32, 6)),
        )

        def flat(m):
            return m[:, :, :].rearrange("p a b -> p (a b)")

        def pads_and_halo(m):
            nc.gpsimd.memset(m[0:64, 0, :], 0.0)
            nc.gpsimd.memset(m[64:128, FR - 1, :], 0.0)
            nc.gpsimd.memset(m[:, :, 0:2], 0.0)
            nc.gpsimd.memset(m[:, :, FC - 2:FC], 0.0)
            nc.gpsimd.dma_start(out=m[0:64, FR - 1, :], in_=m[64:128, 1, :])
            nc.gpsimd.dma_start(out=m[64:128, 0, :], in_=m[0:64, 96, :])

        # ---- build X frame (bf16) ----
        X = xpool.tile([128, FR, FC], bf16, tag="X")
        nc.gpsimd.memset(X[:, :, 0:2], 0.0)
        nc.gpsimd.memset(X[:, :, FC - 2:FC], 0.0)
        nc.gpsimd.memset(X[0:64, 0, :], 0.0)
        nc.gpsimd.memset(X[64:128, FR - 1, :], 0.0)
        for k in range(8):
            xs = stg.tile([128, 12, 192], f32, tag="xs")
            src_t = x_d[:, 12 * k * 192:(12 * k + 12) * 192]
            src_b = x_d[:, (96 + 12 * k) * 192:(96 + 12 * k + 12) * 192]
            nc.sync.dma_start(
                out=xs[0:64, :, :], in_=src_t.rearrange("p (r c) -> p r c", c=192))
            nc.sync.dma_start(
                out=xs[64:128, :, :], in_=src_b.rearrange("p (r c) -> p r c", c=192))
            nc.scalar.copy(X[:, 1 + 12 * k:13 + 12 * k, 2:194], xs[:, :, :])
        nc.gpsimd.dma_start(out=X[0:64, FR - 1, :], in_=X[64:128, 1, :])
        nc.gpsimd.dma_start(out=X[64:128, 0, :], in_=X[0:64, 96, :])

        def conv_taps(ps, wsb, inmap, q, n):
            fin = flat(inmap)
            for t in range(9):
                nc.tensor.matmul(
                    ps[0:64, :n], wsb[0:64, t, :], fin[0:64, q + dl[t]:q + dl[t] + n],
                    start=(t == 0), stop=(t == 8), skip_group_check=True)
                nc.tensor.matmul(
                    ps[64:128, :n], wsb[64:128, t, :], fin[64:128, q + dl[t]:q + dl[t] + n],
                    start=(t == 0), stop=(t == 8), skip_group_check=True,
                    tile_position=(64, 64))

        def da_stage(inmap, kdsb, cwsb, kvsb, cb_col, att_col, outmap):
            fout = flat(outmap)
            fin = flat(inmap)
            att_ap = bias[:, att_col:att_col + 1]
            cb_ap = bias[:, cb_col:cb_col + 1]
            for j, (q, n) in enumerate(qt):
                t1 = t1p.tile([128, TILE], bf16, tag="t1")
                if j % 9 >= _CACHE.get('dw_pe_mod', 5):
                    # depthwise on DVE: per-partition scalar MAC chain
                    acc = accp.tile([128, TILE], bf16, tag="acc")
                    nc.vector.tensor_scalar_mul(
                        acc[:, :n], fin[:, q + dl[0]:q + dl[0] + n], kvsb[:, 0:1])
                    for t in range(1, 9):
                        nc.vector.scalar_tensor_tensor(
                            acc[:, :n], fin[:, q + dl[t]:q + dl[t] + n],
                            kvsb[:, t:t + 1], acc[:, :n],
                            op0=ALU.mult, op1=ALU.add)
                    nc.scalar.activation(t1[:, :n], acc[:, :n], AF.Prelu,
                                         bias=zero_ap, alpha=alpha_ap)
                else:
                    pa = psA.tile([128, TILE], f32, tag="psA")
                    conv_taps(pa, kdsb, inmap, q, n)
                    nc.scalar.activation(t1[:, :n], pa[:, :n], AF.Prelu,
                                         bias=zero_ap, alpha=alpha_ap)
                pb = psB.tile([128, TILE], f32, tag="psB")
                nc.tensor.matmul(pb[0:64, :n], cwsb[0:64, :], t1[0:64, :n],
                                 skip_group_check=True)
                nc.tensor.matmul(pb[64:128, :n], cwsb[64:128, :], t1[64:128, :n],
                                 skip_group_check=True, tile_position=(64, 64))
                t3 = t3p.tile([128, TILE], bf16, tag="t3")
                nc.vector.scalar_tensor_tensor(
                    t3[:, :n], fin[:, q:q + n], att_ap, pb[:, :n],
                    op0=ALU.mult, op1=ALU.add)
                nc.scalar.activation(fout[:, q:q + n], t3[:, :n], AF.Prelu,
                                     bias=cb_ap, alpha=alpha_ap)
            pads_and_halo(outmap)

        def conv_stage(inmap, wsb, b_col, outmap, leaky, finish=True):
            fout = flat(outmap)
            b_ap = bias[:, b_col:b_col + 1]
            for (q, n) in qt:
                pa = psA.tile([128, TILE], f32, tag="psA")
                conv_taps(pa, wsb, inmap, q, n)
                if leaky:
                    nc.scalar.activation(fout[:, q:q + n], pa[:, :n], AF.Prelu,
                                         bias=b_ap, alpha=alpha_ap)
                else:
                    nc.scalar.activation(fout[:, q:q + n], pa[:, :n], AF.Identity,
                                         bias=b_ap)
            if finish:
                pads_and_halo(outmap)

        # ---- network ----
        O1 = mpool.tile([128, FR, FC], bf16, tag="map")
        da_stage(X, kd1, cw1, kv1, BI_CB1, BI_AT1, O1)

        O2 = mpool.tile([128, FR, FC], bf16, tag="map")
        conv_stage(O1, w1, BI_B1, O2, leaky=True, finish=False)
        # additive upsampled map: O2 interior += a32 expanded 6x6
        o2v = O2[:, 1:97, 2:194].rearrange("p (g r) (cc k) -> p g r cc k", r=6, k=6)
        nc.vector.tensor_add(
            o2v, o2v,
            asm[:, :, :, :].unsqueeze(2).broadcast_to((128, 16, 6, 32, 6)))
        pads_and_halo(O2)

        O3 = mpool.tile([128, FR, FC], bf16, tag="map")
        conv_stage(O2, w2, BI_B2, O3, leaky=False)

        O4 = mpool.tile([128, FR, FC], bf16, tag="map")
        da_stage(O3, kd2, cw2, kv2, BI_CB2, BI_AT2, O4)

        O5 = mpool.tile([128, FR, FC], bf16, tag="map")
        conv_stage(O4, w3, BI_B3, O5, leaky=False, finish=False)

        # ---- residual + store ----
        for k in range(8):
            xs = stg.tile([128, 12, 192], f32, tag="xs")
            nc.sync.dma_start(
                out=xs[0:64, :, :],
                in_=x_d[:, 12 * k * 192:(12 * k + 12) * 192]
                .rearrange("p (r c) -> p r c", c=192))
            nc.sync.dma_start(
                out=xs[64:128, :, :],
                in_=x_d[:, (96 + 12 * k) * 192:(96 + 12 * k + 12) * 192]
                .rearrange("p (r c) -> p r c", c=192))
            ot = outp.tile([128, 12, 192], f32, tag="ot")
            nc.vector.tensor_add(ot[:, :, :], O5[:, 1 + 12 * k:13 + 12 * k, 2:194],
                                 xs[:, :, :])
            nc.sync.dma_start(
                out=y_d[:, 12 * k * 192:(12 * k + 12) * 192]
                .rearrange("p (r c) -> p r c", c=192),
                in_=ot[0:64, :, :])
            nc.sync.dma_start(
                out=y_d[:, (96 + 12 * k) * 192:(96 + 12 * k + 12) * 192]
                .rearrange("p (r c) -> p r c", c=192),
                in_=ot[64:128, :, :])

    nc.compile()
    return nc


def _leaky_np(v):
    return np.where(v >= 0, v, ALPHA * v)


def _host_precompute(x, d, p):
    """Build per-core input maps. p: dict of raw weight arrays."""
    d = d.astype(np.float64)
    kern = {}
    att = {}
    for i in (1, 2):
        kw1, kw2 = p[f'da{i}_kw1'].astype(np.float64), p[f'da{i}_kw2'].astype(np.float64)
        ca1, ca2 = p[f'da{i}_ca1'].astype(np.float64), p[f'da{i}_ca2'].astype(np.float64)
        kern[i] = _leaky_np(d @ kw1.T) @ kw2.T          # (B, 576) [c*9+t]
        z = _leaky_np(d @ ca1.T) @ ca2.T
        att[i] = 1.0 / (1.0 + np.exp(-z))               # (B, 64)
    a32 = _leaky_np(d @ p['add_w1'].astype(np.float64).T) @ \
        p['add_w2'].astype(np.float64).T                # (B, 1024)

    cidx = np.arange(128) % 64

    def convw(w):  # (O, C, 3, 3) -> (128, 9, 64) [p, t, o]
        wt = w.transpose(1, 2, 3, 0).reshape(64, 9, 64)  # [c, t, o]
        return np.ascontiguousarray(wt[cidx]).astype(BF16)

    def cwm(w):    # (O, C) -> (128, 64) [p, o]
        return np.ascontiguousarray(w.T[cidx]).astype(BF16)

    w1 = convw(p['conv1_w'])
    w2 = convw(p['conv2_w'])
    w3 = convw(p['conv3_w'])
    cw1 = cwm(p['da1_cw'])
    cw2 = cwm(p['da2_cw'])

    eye = (np.arange(64)[:, None] == np.arange(64)[None, :]).astype(np.float32)
    maps = []
    for b in range(B):
        kd = {}
        for i in (1, 2):
            kc = kern[i][b].reshape(64, 9)               # [c, t]
            kdl = np.einsum('ct,cm->ctm', kc, eye)       # [c, t, m] diag
            kd[i] = np.ascontiguousarray(kdl[cidx]).astype(BF16)
        bias = np.zeros((128, 9), np.float32)
        bias[:, BI_B1] = p['conv1_b'][cidx]
        bias[:, BI_B2] = p['conv2_b'][cidx]
        bias[:, BI_B3] = p['conv3_b'][cidx]
        bias[:, BI_CB1] = p['da1_cb'][cidx]
        bias[:, BI_CB2] = p['da2_cb'][cidx]
        bias[:, BI_AT1] = att[1][b][cidx]
        bias[:, BI_AT2] = att[2][b][cidx]
        bias[:, BI_AL] = ALPHA
        kvs = {i: np.ascontiguousarray(kern[i][b].reshape(64, 9)[cidx]).astype(np.float32)
               for i in (1, 2)}
        a = a32[b].reshape(32, 32)
        a32s = np.zeros((128, 512), np.float32)
        a32s[0:64] = a[0:16].reshape(512)
        a32s[64:128] = a[16:32].reshape(512)
        maps.append(dict(
            x=np.ascontiguousarray(x[b].reshape(C, HW)).astype(np.float32),
            w1=w1, w2=w2, w3=w3, kd1=kd[1], kd2=kd[2], cw1=cw1, cw2=cw2,
            bias=bias, a32s=a32s.astype(np.float32),
            kv1=kvs[1], kv2=kvs[2]))
    return maps


def kernel(**inputs):
    from concourse.bass_utils import run_bass_kernel_spmd

    x = np.asarray(inputs['x'], np.float32)
    d = np.asarray(inputs['d'], np.float32)
    in_maps = _host_precompute(x, d, inputs)

    if 'nc' not in _CACHE:
        _CACHE['nc'] = _build_nc()
    nc = _CACHE['nc']

    res = run_bass_kernel_spmd(nc, in_maps, list(range(B)))
    out = np.stack([np.asarray(res.results[i]['y'], np.float32).reshape(C, H, W)
                    for i in range(B)])
    return out
